# revision 1
# baseline (speedup 1.0000x reference)
"""Cross multi-headed attention (B=4, L=2048, E=512, H=8, dh=64) on 8 trn2 cores.

Sharding: 8 cores = 4 batches x 2 head-groups (4 heads each). Each core:
  - projects its batch's q/k/v against its head-group's weight columns,
  - computes flash-style attention for its 4 heads (full kv length),
  - computes a partial output projection (its heads' contribution).
Host sums the two partials per batch (TP unshard); bo is added on-device by
the even core of each pair (odd cores receive a zero bo input).

Per-core kernel layout (all matmul operands fp16, accumulation fp32):
  stage 1: PE-transpose inputs k/v/q -> [k, l] tiles; project to
    khT/qhT pair-packed [128=2*64dh, 2048] and vfat [128, 4*(64+1)]
    tiles ([Vh | ones] per head: the ones column yields the softmax
    denominator Z during the ctx matmul).
  stage 2 (per 512-wide q chunk, per head pair): S^T tiles [128lk, 2x512]
    via K=64 matmuls (head B uses base-partition-64 implicit row tiling),
    exp via ScalarE (scale=1/sqrt(dh) folded in, no max subtraction:
    scores are O(1) by construction), ctx^T accumulated over 16 lk tiles,
    1/Z broadcast via K=1 ones matmul, scaled ctx^T -> output projection.
"""

import numpy as np
from contextlib import ExitStack

import concourse.bass as bass
import concourse.tile as tile
import concourse.mybir as mybir
import concourse.masks as masks
from concourse.bass_utils import run_bass_kernel_spmd

fp32 = mybir.dt.float32
fp16 = mybir.dt.float16
AF = mybir.ActivationFunctionType

B, L, E = 4, 2048, 512
H, DH = 8, 64
DG = 256          # head-group width (4 heads x 64)
SCALE = 1.0 / 8.0  # 1/sqrt(64)
NKT = 4           # k tiles of 128 over E=512
NCH = 4           # l chunks of 512
CH = 512
NLB = 16          # l blocks of 128
N_CORES = 8

_wsplit_counter = [0]


def _split_sync_waits(nc, limit=1):
    """This walrus build accepts only one semaphore wait per instruction.

    Move excess waits onto same-engine NoOps placed directly before the
    offending instruction (engine streams are in-order, so this is
    equivalent).
    """
    n = 0
    for f in nc.m.functions:
        for bb in f.blocks:
            out = []
            changed = False
            for inst in bb.instructions:
                si = inst.sync_info
                if si is not None and si.on_wait and len(si.on_wait) > limit:
                    waits = list(si.on_wait)
                    excess, keep = waits[:-limit], waits[-limit:]
                    for i in range(0, len(excess), limit):
                        chunk = excess[i : i + limit]
                        _wsplit_counter[0] += 1
                        out.append(
                            mybir.InstNoOp(
                                name=f"I-wsplit-{_wsplit_counter[0]}",
                                ins=[],
                                outs=[],
                                engine=inst.engine,
                                sync_info=mybir.SyncInfo(
                                    on_wait=chunk, on_update=[]
                                ),
                            )
                        )
                        n += 1
                    inst.sync_info = mybir.SyncInfo(
                        on_wait=keep, on_update=si.on_update
                    )
                    changed = True
                out.append(inst)
            if changed:
                bb.instructions = out
    return n


class _SplitDrainTileContext(tile.TileContext):
    """Tail drain emitting one wait_ge per semaphore (vs one mega-drain)."""

    def _drain_and_barrier(self, tick_clock, wait_clock):
        gc = tick_clock.global_clock
        assert self.sems is not None
        for proc_idx, sem in self.sems.allocated().items():
            try:
                val = gc[proc_idx]
            except (IndexError, OverflowError):
                val = 0
            if val > 0:
                self.nc.sync.wait_ge(sem, val)
        self.nc.sync.drain()
        self.nc.all_engine_barrier()
        popped = self.nc._tile_sem_poison_stack.pop()
        assert popped is self._sem_poison
        self.nc.clear_and_free_semaphores(list(self.sems.allocated().values()))
        self.nc.all_engine_barrier()


def build_nc():
    nc = bass.Bass(
        "TRN2", target_bir_lowering=False, debug=False, num_devices=N_CORES
    )
    xq = nc.declare_dram_parameter("xq", [L, E], fp32, isOutput=False)
    xk = nc.declare_dram_parameter("xk", [L, E], fp32, isOutput=False)
    xv = nc.declare_dram_parameter("xv", [L, E], fp32, isOutput=False)
    wq = nc.declare_dram_parameter("wq", [E, DG], fp32, isOutput=False)
    wk = nc.declare_dram_parameter("wk", [E, DG], fp32, isOutput=False)
    wv = nc.declare_dram_parameter("wv", [E, DG], fp32, isOutput=False)
    wo = nc.declare_dram_parameter("wo", [DG, E], fp32, isOutput=False)
    bq = nc.declare_dram_parameter("bq", [DG], fp32, isOutput=False)
    bk = nc.declare_dram_parameter("bk", [DG], fp32, isOutput=False)
    bv = nc.declare_dram_parameter("bv", [DG], fp32, isOutput=False)
    bo = nc.declare_dram_parameter("bo", [E], fp32, isOutput=False)
    out = nc.declare_dram_parameter("out", [L, E], fp32, isOutput=True)

    with _SplitDrainTileContext(nc) as tc, ExitStack() as ctx:
        # ---- persistent tiles -------------------------------------------
        pp = ctx.enter_context(tc.tile_pool(name="persist", bufs=1))

        ones_row = pp.tile([1, 128], fp16, tag="ones_row", name="ones_row")
        nc.vector.memset(ones_row[:], 1.0)

        ident = pp.tile([128, 128], fp32, tag="ident", name="ident")
        masks.make_identity(nc, ident[:])

        qhT = [pp.tile([128, L], fp16, tag=f"qhT{p}", name=f"qhT{p}") for p in range(2)]
        khT = [pp.tile([128, L], fp16, tag=f"khT{p}", name=f"khT{p}") for p in range(2)]
        vfat = [pp.tile([128, 4 * 65], fp16, tag=f"vfat{t}", name=f"vfat{t}") for t in range(NLB)]
        for t in range(NLB):
            oc = vfat[t][:].rearrange("p (h c) -> p h c", c=65)[:, :, 64:65]
            nc.vector.memset(oc, 1.0)

        # ---- stages: k/v/q projection interleaved with attention --------
        # PSUM budget (8 banks): s 2x2 + ctx 2x1 + scratch 2x1
        with (
            tc.tile_pool(name="spool", bufs=2, space="PSUM") as sps,
            tc.tile_pool(name="cpool", bufs=1, space="PSUM") as cps,
            tc.tile_pool(name="scratch", bufs=2, space="PSUM") as scr,
            tc.tile_pool(name="xsb", bufs=6) as xsb,
            tc.tile_pool(name="xxt", bufs=2) as xxt,
            tc.tile_pool(name="ppool", bufs=9) as ppool,
            tc.tile_pool(name="cspool", bufs=2) as cspool,
            tc.tile_pool(name="zsb", bufs=2) as zsb,
            tc.tile_pool(name="outsb", bufs=2) as outsb,
        ):
            def load_chunk(xdram, c, split=1):
                # split consolidated DMAs per 512-row chunk:
                # part s covers lbs [s*4//split, ...); within a part,
                # tile[p, i*512+e] = x[c*512+(lb0+i)*128+p, e]
                nlb = 4 // split
                parts = []
                for s in range(split):
                    st = xsb.tile(
                        [128, nlb * E], fp32, tag="xstage", name="xstage"
                    )
                    r0 = c * CH + s * nlb * 128
                    src = xdram[r0 : r0 + nlb * 128, :].rearrange(
                        "(b p) e -> p b e", p=128
                    )
                    nc.sync.dma_start(
                        st[:].rearrange("p (b e) -> p b e", b=nlb), src
                    )
                    parts.append(st)
                return parts

            def transpose_chunk(xdram, c, sts=None):
                # PE-transpose fp32 input blocks into PSUM; the copy out of
                # PSUM converts to fp16 on DVE.
                if sts is None:
                    sts = load_chunk(xdram, c)
                nlb = 4 // len(sts)
                xTall = xxt.tile([128, NKT * CH], fp16, tag="xT", name="xT")
                for lb in range(4):
                    st4 = sts[lb // nlb]
                    tp = scr.tile([128, E], fp32, tag="scratch", name="tp")
                    for kt in range(NKT):
                        col = (lb % nlb) * E + kt * 128
                        nc.tensor.transpose(
                            tp[:, kt * 128 : (kt + 1) * 128],
                            st4[:, col : col + 128],
                            ident[:],
                        )
                    dst = xTall[:].rearrange(
                        "p (k b l) -> p k b l", k=NKT, b=4
                    )[:, :, lb, :]
                    src = tp[:].rearrange("p (k l) -> p k l", k=NKT)
                    nc.vector.tensor_copy(dst, src)
                xT = [
                    xTall[:, kt * CH : (kt + 1) * CH] for kt in range(NKT)
                ]
                return xT

            def qk_chunk(kind, c, sts=None):
                xT = transpose_chunk(xq if kind == "q" else xk, c, sts)
                wt = w16["wq" if kind == "q" else "wk"]
                bias = bq_sb if kind == "q" else bk_sb
                dst = qhT if kind == "q" else khT
                for p in range(2):
                    pps = scr.tile([128, CH], fp32, tag="scratch", name="pps")
                    for kt in range(NKT):
                        nc.tensor.matmul(
                            pps[:],
                            wt[kt][:, p * 128 : (p + 1) * 128],
                            xT[kt][:],
                            start=(kt == 0),
                            stop=(kt == NKT - 1),
                        )
                    nc.vector.tensor_scalar_add(
                        dst[p][:, c * CH : (c + 1) * CH],
                        pps[:],
                        bias[:, p : p + 1],
                    )

            def v_chunk(c, sts=None):
                xT = transpose_chunk(xv, c, sts)
                for lb in range(4):
                    vps = scr.tile([128, DG], fp32, tag="scratch", name="vps")
                    for kt in range(NKT):
                        nc.tensor.matmul(
                            vps[:],
                            xT[kt][:, lb * 128 : (lb + 1) * 128],
                            w16["wv"][kt][:],
                            start=(kt == 0),
                            stop=False,
                        )
                    nc.tensor.matmul(
                        vps[:], ones_row[0:1, :], bv16[:], start=False, stop=True
                    )
                    t = c * 4 + lb
                    dst = vfat[t][:].rearrange("p (h c) -> p h c", c=65)[
                        :, :, 0:64
                    ]
                    src = vps[:].rearrange("p (h c) -> p h c", c=64)
                    nc.vector.tensor_copy(dst, src)

            def s_exp(c, p, t):
                lq0 = c * CH
                sp = sps.tile([128, 2 * CH], fp32, tag="s", name="s")
                for ab in range(2):
                    nc.tensor.matmul(
                        sp[:, ab * CH : (ab + 1) * CH],
                        khT[p][
                            ab * 64 : (ab + 1) * 64, t * 128 : (t + 1) * 128
                        ],
                        qhT[p][ab * 64 : (ab + 1) * 64, lq0 : lq0 + CH],
                        start=True,
                        stop=True,
                        tile_position=(ab * 64, 0),
                    )
                pT = ppool.tile([128, 2 * CH], fp16, tag="pT", name="pT")
                nc.scalar.activation(pT[:], sp[:], AF.Exp, scale=SCALE)
                return pT

            def ctx_mms(c, p, ctx_ps, t, pT):
                for ab in range(2):
                    lh = 2 * p + ab
                    nc.tensor.matmul(
                        ctx_ps[ab][:],
                        vfat[t][:, lh * 65 : (lh + 1) * 65],
                        pT[:, ab * CH : (ab + 1) * CH],
                        start=(t == 0),
                        stop=(t == NLB - 1),
                    )

            def att_segment(c, p, ctx_ps, t0, t1):
                for t in range(t0, t1):
                    pT = s_exp(c, p, t)
                    ctx_mms(c, p, ctx_ps, t, pT)

            def att_tail(c, p, ctx_ps, ctxS):
                # batch the DVE z-prep for both heads ahead of the PE
                # broadcast matmuls so PE waits once, not twice
                zr16s = []
                for ab in range(2):
                    zr32 = zsb.tile([1, CH], fp32, tag=f"zr32_{ab}", name="zr32")
                    nc.vector.reciprocal(zr32[:], ctx_ps[ab][64:65, :])
                    zr16 = zsb.tile([1, CH], fp16, tag=f"zr16_{ab}", name="zr16")
                    nc.vector.tensor_copy(zr16[:], zr32[:])
                    zr16s.append(zr16)
                zb_pss = []
                for ab in range(2):
                    zb_ps = scr.tile([64, CH], fp32, tag="scratch", name="zb")
                    nc.tensor.matmul(
                        zb_ps[:],
                        ones_row[0:1, 0:64],
                        zr16s[ab][:],
                        start=True,
                        stop=True,
                    )
                    zb_pss.append(zb_ps)
                for ab in range(2):
                    lh = 2 * p + ab
                    zb16 = zsb.tile([64, CH], fp16, tag=f"zb16_{ab}", name="zb16")
                    nc.vector.tensor_copy(zb16[:], zb_pss[ab][:])
                    cs = cspool.tile(
                        [64, CH], fp16, tag=f"ctxS{lh}", name=f"ctxS{lh}"
                    )
                    nc.vector.tensor_mul(cs[:], ctx_ps[ab][0:64, :], zb16[:])
                    ctxS[lh] = cs

            def out_proj(c, ctxS):
                lq0 = c * CH
                for j in range(4):
                    op_ps = scr.tile([128, E], fp32, tag="scratch", name="op")
                    for lh in range(4):
                        nc.tensor.matmul(
                            op_ps[:],
                            ctxS[lh][:, j * 128 : (j + 1) * 128],
                            wo16[lh][:],
                            start=(lh == 0),
                            stop=False,
                        )
                    nc.tensor.matmul(
                        op_ps[:],
                        ones_row[0:1, :],
                        bo16[:],
                        start=False,
                        stop=True,
                    )
                    osb = outsb.tile([128, E], fp32, tag="osb", name="osb")
                    nc.vector.tensor_copy(osb[:], op_ps[:])
                    row0 = lq0 + j * 128
                    nc.sync.dma_start(out[row0 : row0 + 128, :], osb[:])

            def alloc_ctx():
                return {
                    0: cps.tile([65, CH], fp32, tag="ctxA", name="ctxA"),
                    1: cps.tile([65, CH], fp32, tag="ctxB", name="ctxB"),
                }

            # Preload chunk-0 inputs FIRST so their DMAs lead the SP queue;
            # weight/bias staging DMAs queue up behind them and their DVE
            # converts overlap the PE transposes of k0/q0.
            k0_sts = load_chunk(xk, 0, split=2)
            q0_sts = load_chunk(xq, 0, split=2)

            # weights (fp32 staged -> fp16); one consolidated stage DMA per
            # weight, ordered by first use: wk/wq before v0, wv before wo.
            def stage_w(name, dram):
                st = pp.tile(
                    [128, NKT * DG], fp32, tag=f"wst_{name}", name="wst"
                )
                nc.sync.dma_start(
                    st[:].rearrange("p (k d) -> p k d", k=NKT),
                    dram[:, :].rearrange("(k p) d -> p k d", p=128),
                )
                tiles = []
                for kt in range(NKT):
                    t16 = pp.tile(
                        [128, DG], fp16, tag=f"{name}16_{kt}",
                        name=f"{name}16_{kt}",
                    )
                    nc.vector.tensor_copy(
                        t16[:], st[:, kt * DG : (kt + 1) * DG]
                    )
                    tiles.append(t16)
                return tiles

            w16 = {}
            w16["wk"] = stage_w("wk", wk)
            w16["wq"] = stage_w("wq", wq)
            bq_sb = pp.tile([128, 2], fp32, tag="bq_sb", name="bq_sb")
            nc.sync.dma_start(bq_sb[:], bq.rearrange("(b p) -> p b", p=128))
            bk_sb = pp.tile([128, 2], fp32, tag="bk_sb", name="bk_sb")
            nc.sync.dma_start(bk_sb[:], bk.rearrange("(b p) -> p b", p=128))

            v0_sts = load_chunk(xv, 0)
            w16["wv"] = stage_w("wv", wv)
            bv_st = pp.tile([1, DG], fp32, tag="bv_st", name="bv_st")
            nc.sync.dma_start(bv_st[:], bv.rearrange("(a c) -> a c", a=1))
            bv16 = pp.tile([1, DG], fp16, tag="bv16", name="bv16")
            nc.vector.tensor_copy(bv16[:], bv_st[:])

            k1_sts = load_chunk(xk, 1)

            wost = pp.tile([64, 4 * E], fp32, tag="wost", name="wost")
            nc.sync.dma_start(
                wost[:].rearrange("p (h e) -> p h e", h=4),
                wo[:, :].rearrange("(h p) e -> p h e", p=64),
            )
            wo16 = []
            for lh in range(4):
                t16 = pp.tile([64, E], fp16, tag=f"wo16_{lh}", name="wo16")
                nc.vector.tensor_copy(t16[:], wost[:, lh * E : (lh + 1) * E])
                wo16.append(t16)
            bo_st = pp.tile([1, E], fp32, tag="bo_st", name="bo_st")
            nc.sync.dma_start(bo_st[:], bo.rearrange("(a c) -> a c", a=1))
            bo16 = pp.tile([1, E], fp16, tag="bo16", name="bo16")
            nc.vector.tensor_copy(bo16[:], bo_st[:])

            # chunk 0: S/exp for t=0..3 needs only k0+q0 — emit before v0 so
            # ACT starts as early as possible; their ctx matmuls wait for v0.
            qk_chunk("k", 0, k0_sts)
            qk_chunk("q", 0, q0_sts)
            ctx_p0 = alloc_ctx()
            pTs = [s_exp(0, 0, t) for t in range(4)]
            # pair-1 exps for t=0..3 also only need k0+q0 — emit them early to
            # fill ACT idle during the k/v phase; their ctx matmuls run when
            # pair-1's PSUM accumulators open up (pT tiles held until then).
            held_p1 = {t: s_exp(0, 1, t) for t in range(4)}
            v_chunk(0, v0_sts)
            for t in range(4):
                ctx_mms(0, 0, ctx_p0, t, pTs[t])
            for kc in range(1, NCH):
                qk_chunk("k", kc, k1_sts if kc == 1 else None)
                pTs = [s_exp(0, 0, t) for t in range(4 * kc, 4 * kc + 4)]
                v_chunk(kc)
                for i, t in enumerate(range(4 * kc, 4 * kc + 4)):
                    ctx_mms(0, 0, ctx_p0, t, pTs[i])
            ctxS0 = {}
            att_tail(0, 0, ctx_p0, ctxS0)
            ctx_p1 = alloc_ctx()
            for t in range(4):
                ctx_mms(0, 1, ctx_p1, t, held_p1[t])
            att_segment(0, 1, ctx_p1, 4, NLB)
            # prefetch next q chunk while ACT chews on this chunk's exps
            qk_chunk("q", 1)
            att_tail(0, 1, ctx_p1, ctxS0)

            # out_proj(c) is deferred into chunk c+1's ACT-bound window
            prev_ctxS = ctxS0
            for c in range(1, NCH):
                ctxS = {}
                ctx_p0c = alloc_ctx()
                att_segment(c, 0, ctx_p0c, 0, NLB)
                out_proj(c - 1, prev_ctxS)
                att_tail(c, 0, ctx_p0c, ctxS)
                ctx_p1c = alloc_ctx()
                att_segment(c, 1, ctx_p1c, 0, NLB)
                if c + 1 < NCH:
                    qk_chunk("q", c + 1)
                att_tail(c, 1, ctx_p1c, ctxS)
                prev_ctxS = ctxS
            out_proj(NCH - 1, prev_ctxS)

    _split_sync_waits(nc, limit=1)
    return nc


_CACHED_NC = None


def _get_nc():
    global _CACHED_NC
    if _CACHED_NC is None:
        _CACHED_NC = build_nc()
    return _CACHED_NC


def make_in_maps(query, key, value, wq, bq, wk, bk, wv, bv, wo, bo):
    query = np.asarray(query, dtype=np.float32)
    key = np.asarray(key, dtype=np.float32)
    value = np.asarray(value, dtype=np.float32)
    wq = np.asarray(wq, dtype=np.float32)
    wk = np.asarray(wk, dtype=np.float32)
    wv = np.asarray(wv, dtype=np.float32)
    wo = np.asarray(wo, dtype=np.float32)
    bq = np.asarray(bq, dtype=np.float32)
    bk = np.asarray(bk, dtype=np.float32)
    bv = np.asarray(bv, dtype=np.float32)
    bo = np.asarray(bo, dtype=np.float32)
    zeros_bo = np.zeros_like(bo)
    in_maps = []
    for core in range(N_CORES):
        b, g = core // 2, core % 2
        cols = slice(g * DG, (g + 1) * DG)
        in_maps.append(
            {
                "xq": np.ascontiguousarray(query[b]),
                "xk": np.ascontiguousarray(key[b]),
                "xv": np.ascontiguousarray(value[b]),
                "wq": np.ascontiguousarray(wq[:, cols]),
                "wk": np.ascontiguousarray(wk[:, cols]),
                "wv": np.ascontiguousarray(wv[:, cols]),
                "wo": np.ascontiguousarray(wo[cols, :]),
                "bq": np.ascontiguousarray(bq[cols]),
                "bk": np.ascontiguousarray(bk[cols]),
                "bv": np.ascontiguousarray(bv[cols]),
                "bo": bo if g == 0 else zeros_bo,
            }
        )
    return in_maps


def run(in_maps, **kwargs):
    nc = _get_nc()
    return run_bass_kernel_spmd(nc, in_maps, list(range(N_CORES)), **kwargs)


def kernel(query, key, value, wq, bq, wk, bk, wv, bv, wo, bo):
    in_maps = make_in_maps(query, key, value, wq, bq, wk, bk, wv, bv, wo, bo)
    res = run(in_maps)
    out = np.empty((B, L, E), dtype=np.float32)
    for b in range(B):
        out[b] = res.results[2 * b]["out"] + res.results[2 * b + 1]["out"]
    return out

